# revision 67
# baseline (speedup 1.0000x reference)
"""Causal GQA attention block (B=2, T=2048, C=2048, H=16, HKV=4, D=128, RoPE)
on 8 Trainium2 NeuronCores.

Sharding: core c handles batch b = c//4 and kv-group g = c%4 (4 q heads +
1 kv head per core).  The output projection is row-parallel: each core
produces a partial [T, C] contribution; the host sums the 4 partials per
batch.

Device-side design:
  - x is host-transposed to xT [C, T] so projections contract over the
    partition dim without any on-device transpose.
  - q/k are produced in head-transposed layout [D, T]; RoPE is applied
    there using a host-side even/odd permutation folded into wq/wk plus
    partition-swapped multiplies (out = q*cos + swap(q)*sin).
  - scores are computed transposed (S.T tiles [s, t]) so exp'd tiles feed
    the attn@v matmul directly as the moving operand; the softmax
    denominator comes from a ones-vector matmul accumulated alongside.
  - no max-subtraction in softmax: scores here are O(6), exp is safe in
    fp32, and the result is mathematically identical to the reference.
  - jt0 projections run k-outer (consuming streamed weight chunks), later
    chunks run output-major over an SBUF-cached x chunk set with a
    6-bank rotating PSUM pool so RoPE overlaps accumulation.

Matmul precision mode (env BASS_ATTN_MODE): "f32r" (default; fp32 storage,
float32r matmuls, ~1.4e-4 matmul rel err at full speed), "f32" (exact, 4x
slower), "bf16" (bf16 storage).
"""

import os
from contextlib import ExitStack

import numpy as np

import concourse.bass as bass
import concourse.tile as tile
from concourse import bacc, mybir
from concourse.bass_utils import run_bass_kernel_spmd
from concourse.masks import make_identity

# problem constants
B, T, C = 2, 2048, 2048
H, HKV, D = 16, 4, 128
GROUP = H // HKV           # 4 q heads per kv head
THETA = 1000000.0
SCALE = D ** -0.5

P = 128                    # partitions
TCH = 512                  # t-chunk (matmul moving free dim)
NJT = T // TCH             # 4 t-chunks
NK = C // P                # 16 contraction tiles
NH = GROUP                 # 4 local q heads
NST = T // P               # 16 s-tiles
N_CORES = 8

F32 = mybir.dt.float32


def _sb_dt(mode):
    if mode == "bf16":
        return mybir.dt.bfloat16
    if mode == "f32r":
        return mybir.dt.float32r
    return F32


def _np_dt(mode):
    if mode == "bf16":
        import ml_dtypes
        return ml_dtypes.bfloat16
    return np.float32


def build_program(mode="f32r", phases="ABC", variant=""):
    """Build and compile the per-core Bass program. Returns nc.

    phases/variant are diagnostic knobs for timeline bisection; production
    callers use the defaults.
    """
    sb_dt = _sb_dt(mode)

    nc = bacc.Bacc("TRN2", target_bir_lowering=False, debug=False)

    xT_d = nc.dram_tensor("xT", [C, T], sb_dt, kind="ExternalInput").ap()
    wq_d = nc.dram_tensor("wqT", [C, NH * D], sb_dt, kind="ExternalInput").ap()
    wk_d = nc.dram_tensor("wkT", [C, D], sb_dt, kind="ExternalInput").ap()
    wv_d = nc.dram_tensor("wvT", [C, D], sb_dt, kind="ExternalInput").ap()
    wo_d = nc.dram_tensor("woT", [NH * D, C], sb_dt, kind="ExternalInput").ap()
    cos_d = nc.dram_tensor("cosT", [P, T], F32, kind="ExternalInput").ap()
    sin_d = nc.dram_tensor("sinT", [P, T], F32, kind="ExternalInput").ap()
    msk_d = nc.dram_tensor("mskT", [P, 4 * TCH], sb_dt, kind="ExternalInput").ap()
    ones_d = nc.dram_tensor("ones", [P, 1], sb_dt, kind="ExternalInput").ap()
    y_d = nc.dram_tensor("y", [T, C], F32, kind="ExternalOutput").ap()

    with tile.TileContext(nc) as tc, ExitStack() as ctx:
        wpool = ctx.enter_context(tc.tile_pool(name="weights", bufs=1))
        tpool = ctx.enter_context(tc.tile_pool(name="tables", bufs=1))
        state = ctx.enter_context(tc.tile_pool(name="state", bufs=1))

        # wq in 2-k chunks on the scalar queue (consumed in k order by the
        # k-outer jt0 projections); wk/wv whole on gpsimd
        wq_sb = wpool.tile([P, NK, NH * D], sb_dt, tag="wq")
        wk_sb = wpool.tile([P, NK, D], sb_dt, tag="wk")
        wv_sb = wpool.tile([P, NK, D], sb_dt, tag="wv")
        for k2 in range(NK // 2):
            nc.scalar.dma_start(
                wq_sb[:, 2 * k2:2 * k2 + 2, :],
                wq_d[2 * k2 * P:(2 * k2 + 2) * P, :].rearrange(
                    "(ko p) o -> p ko o", p=P))
        nc.gpsimd.dma_start(wk_sb[:], wk_d.rearrange("(ko p) o -> p ko o", p=P))
        nc.gpsimd.dma_start(wv_sb[:], wv_d.rearrange("(ko p) o -> p ko o", p=P))

        cos_sb = tpool.tile([P, T], F32, tag="cos")
        sin_sb = tpool.tile([P, T], F32, tag="sin")
        for jt in range(NJT):
            sl = slice(jt * TCH, (jt + 1) * TCH)
            nc.gpsimd.dma_start(cos_sb[:, sl], cos_d[:, sl])
            nc.gpsimd.dma_start(sin_sb[:, sl], sin_d[:, sl])
        msk_sb = tpool.tile([P, 4 * TCH], sb_dt, tag="msk")
        nc.gpsimd.dma_start(msk_sb[:], msk_d[:])
        ones_sb = tpool.tile([P, 1], sb_dt, tag="ones")
        nc.gpsimd.dma_start(ones_sb[:], ones_d[:])
        ident_sb = tpool.tile([P, P], F32, tag="ident")
        make_identity(nc, ident_sb[:])

        qrot = state.tile([P, NH, T], sb_dt, tag="qrot")
        krot = state.tile([P, T], sb_dt, tag="krot")
        v_sb = state.tile([P, NST, D], sb_dt, tag="v")
        ot_sb = state.tile([P, NH, T], sb_dt, tag="ot")

        # ---------------- Phase A: projections + RoPE -----------------
        with ExitStack() as actx:
          if "A" in phases:
            xpool = actx.enter_context(tc.tile_pool(name="xsub", bufs=1))
            xpool2 = actx.enter_context(tc.tile_pool(name="xsub2", bufs=2))
            ropep = actx.enter_context(tc.tile_pool(name="rope", bufs=2))
            vtp = actx.enter_context(tc.tile_pool(name="vt", bufs=1))
            psA = actx.enter_context(tc.tile_pool(name="psA", bufs=1, space="PSUM"))
            psT = actx.enter_context(tc.tile_pool(name="psT", bufs=2, space="PSUM"))

            def rope(acc_ps, out_ap, jt):
                ch = slice(jt * TCH, (jt + 1) * TCH)
                m1 = ropep.tile([P, TCH], F32, tag="m1")
                m2 = ropep.tile([P, TCH], F32, tag="m2")
                nc.vector.tensor_tensor(
                    m1[:], acc_ps[:], cos_sb[:, ch], mybir.AluOpType.mult)
                nc.vector.tensor_tensor(
                    m2[0:64, :], acc_ps[64:128, :], sin_sb[0:64, ch],
                    mybir.AluOpType.mult)
                nc.vector.tensor_tensor(
                    m2[64:128, :], acc_ps[0:64, :], sin_sb[64:128, ch],
                    mybir.AluOpType.mult)
                nc.vector.tensor_tensor(
                    out_ap, m1[:], m2[:], mybir.AluOpType.add)

            def w_slice(o, k):
                # output index o: 0..3 = q heads, 4 = k, 5 = v
                if o < NH:
                    return wq_sb[:, k, o * D:(o + 1) * D]
                if o == NH:
                    return wk_sb[:, k, :]
                return wv_sb[:, k, :]

            def finish(o, acc, jt):
                if o < NH:
                    rope(acc, qrot[:, o, jt * TCH:(jt + 1) * TCH], jt)
                elif o == NH:
                    rope(acc, krot[:, jt * TCH:(jt + 1) * TCH], jt)
                else:
                    vt = vtp.tile([P, TCH], F32, tag="vt")
                    nc.vector.tensor_copy(vt[:], acc[:])
                    for i in range(TCH // P):
                        pst = psT.tile([P, P], F32, tag="pst")
                        nc.tensor.transpose(pst[:], vt[:, i * P:(i + 1) * P],
                                            ident_sb[:])
                        nc.scalar.copy(v_sb[:, jt * (TCH // P) + i, :], pst[:])

            nacc = 0  # rotating psum bank index
            for jt in range(NJT):
                xs = []
                for k in range(NK):
                    pool_k = xpool2 if k < 6 else xpool
                    xt = pool_k.tile([P, TCH], sb_dt, tag=f"x{k}")
                    nc.sync.dma_start(
                        xt[:],
                        xT_d[k * P:(k + 1) * P, jt * TCH:(jt + 1) * TCH])
                    xs.append(xt)

                if jt == 0:
                    # k-outer: consume weight chunks as they stream in
                    accs = [psA.tile([P, TCH], F32, tag=f"acc{o}", name=f"acc{o}")
                            for o in range(6)]
                    for k in range(NK):
                        for o in range(6):
                            nc.tensor.matmul(
                                accs[o][:], w_slice(o, k), xs[k][:],
                                start=(k == 0), stop=(k == NK - 1))
                    for o in range(6):
                        finish(o, accs[o], jt)
                    nacc = 6
                else:
                    # output-major: RoPE of one output overlaps the next
                    # output's accumulation via the rotating bank pool
                    for o in range(6):
                        acc = psA.tile([P, TCH], F32, tag=f"acc{nacc % 6}",
                                       name=f"accr{nacc % 6}")
                        nacc += 1
                        for k in range(NK):
                            nc.tensor.matmul(
                                acc[:], w_slice(o, k), xs[k][:],
                                start=(k == 0), stop=(k == NK - 1))
                        finish(o, acc, jt)

        # ---------------- Phase B: attention ---------------------------
        # prefetch the output-projection weights during attention so phase C
        # starts without a DMA stall
        wopool = ctx.enter_context(tc.tile_pool(name="wo", bufs=1))
        wo_sb = wopool.tile([P, NH, C], sb_dt, tag="wo")
        for h in range(NH):
            nc.scalar.dma_start(wo_sb[:, h, :], wo_d[h * P:(h + 1) * P, :])

        with ExitStack() as bctx:
          if "B" in phases:
            esp = bctx.enter_context(tc.tile_pool(name="es", bufs=4))
            rcp = bctx.enter_context(tc.tile_pool(name="rc", bufs=2))
            psS = bctx.enter_context(tc.tile_pool(name="psS", bufs=4, space="PSUM"))
            psO = bctx.enter_context(tc.tile_pool(name="psO", bufs=3, space="PSUM"))
            psD = bctx.enter_context(tc.tile_pool(name="psD", bufs=1, space="PSUM"))

            for jt in range(NJT):
                for h in range(NH):
                    njs = 4 * jt + 4
                    ot_ps = psO.tile([P, TCH], F32, tag="ot")
                    dn_ps = psD.tile([1, TCH], F32, tag="dn")
                    qch = qrot[:, h, jt * TCH:(jt + 1) * TCH]

                    def emit_pv(es, js, njs=njs, ot_ps=ot_ps, dn_ps=dn_ps):
                        nc.tensor.matmul(
                            ot_ps[:], v_sb[:, js, :], es[:],
                            start=(js == 0), stop=(js == njs - 1))
                        if variant != "noden":
                            nc.tensor.matmul(
                                dn_ps[:], ones_sb[:, 0:1], es[:],
                                start=(js == 0), stop=(js == njs - 1))

                    pend = []  # deferred two steps to hide exp latency
                    for js in range(njs):
                        s_ps = psS.tile([P, TCH], F32, tag="s")
                        nc.tensor.matmul(
                            s_ps[:], krot[:, js * P:(js + 1) * P], qch,
                            start=True, stop=True)
                        es = esp.tile([P, TCH], sb_dt, tag="es")
                        nc.scalar.activation(
                            es[:], s_ps[:], mybir.ActivationFunctionType.Exp)
                        if js // 4 == jt:
                            nc.vector.tensor_tensor(
                                es[:], es[:],
                                msk_sb[:, (js % 4) * TCH:(js % 4 + 1) * TCH],
                                mybir.AluOpType.mult)
                        if len(pend) >= 3:
                            emit_pv(*pend.pop(0))
                        pend.append((es, js))
                    for p_ in pend:
                        emit_pv(*p_)

                    if variant == "noden":
                        nc.vector.tensor_copy(
                            ot_sb[:, h, jt * TCH:(jt + 1) * TCH], ot_ps[:])
                    else:
                        rec = rcp.tile([1, TCH], F32, tag="rec")
                        nc.vector.reciprocal(rec[:], dn_ps[:])
                        rb = rcp.tile([P, TCH], F32, tag="rb")
                        nc.gpsimd.partition_broadcast(rb[:], rec[:])
                        nc.vector.tensor_tensor(
                            ot_sb[:, h, jt * TCH:(jt + 1) * TCH], ot_ps[:],
                            rb[:], mybir.AluOpType.mult)

        # ---------------- Phase C: output projection -------------------
        with ExitStack() as cctx:
          if "C" in phases:
            ypool = cctx.enter_context(tc.tile_pool(name="ysb", bufs=8))
            psC = cctx.enter_context(tc.tile_pool(name="psC", bufs=4, space="PSUM"))

            for tt in range(NST):
                for jc in range(NJT):
                    yp = psC.tile([P, TCH], F32, tag="y")
                    for h in range(NH):
                        nc.tensor.matmul(
                            yp[:],
                            ot_sb[:, h, tt * P:(tt + 1) * P],
                            wo_sb[:, h, jc * TCH:(jc + 1) * TCH],
                            start=(h == 0), stop=(h == NH - 1))
                    ys = ypool.tile([P, TCH], F32, tag="ys")
                    nc.vector.tensor_copy(ys[:], yp[:])
                    nc.sync.dma_start(
                        y_d[tt * P:(tt + 1) * P, jc * TCH:(jc + 1) * TCH],
                        ys[:])

    nc.compile()
    return nc


def host_prep(x, wq, wk, wv, wo, mode="f32r"):
    """Build the 8 per-core input maps (numpy, host-side reshuffles only)."""
    ndt = _np_dt(mode)
    x = np.asarray(x, dtype=np.float32)
    wq = np.asarray(wq, dtype=np.float32)
    wk = np.asarray(wk, dtype=np.float32)
    wv = np.asarray(wv, dtype=np.float32)
    wo = np.asarray(wo, dtype=np.float32)

    # RoPE even/odd grouping permutation within each head
    perm = np.concatenate([np.arange(0, D, 2), np.arange(1, D, 2)])

    # rope tables, transposed layout [d, t], matching reference f32 math
    inv_freq = (1.0 / THETA ** (np.arange(0, D, 2, dtype=np.float32) / D)).astype(np.float32)
    pos = np.arange(T, dtype=np.float32)
    freqs = pos[:, None] * inv_freq[None, :]          # [T, 64] f32
    cos_t = np.cos(freqs).astype(np.float32).T        # [64, T]
    sin_t = np.sin(freqs).astype(np.float32).T        # [64, T]
    cosT = np.concatenate([cos_t, cos_t], axis=0)     # [128, T]
    sinT = np.concatenate([-sin_t, sin_t], axis=0)    # [128, T]

    # diagonal-block causal masks (multiplicative, after exp)
    # pattern r (= js % 4): allow f >= 128*r + p
    f = np.arange(TCH)[None, :]
    p = np.arange(P)[:, None]
    msk = np.concatenate(
        [(f >= P * r + p).astype(np.float32) for r in range(4)], axis=1)

    xTs = [np.ascontiguousarray(x[b].T).astype(ndt) for b in range(B)]

    in_maps = []
    for c in range(N_CORES):
        b, g = divmod(c, GROUP)
        rows = []
        for hh in range(NH):
            h = g * GROUP + hh
            rows.append(wq[h * D + perm, :])
        wq_g = np.concatenate(rows, axis=0) * SCALE          # [512, C]
        wk_g = wk[g * D + perm, :]                           # [128, C]
        wv_g = wv[g * D:(g + 1) * D, :]                      # [128, C]
        wo_g = wo[:, g * NH * D:(g + 1) * NH * D]            # [C, 512]

        in_maps.append({
            "xT": xTs[b],
            "wqT": np.ascontiguousarray(wq_g.T).astype(ndt),
            "wkT": np.ascontiguousarray(wk_g.T).astype(ndt),
            "wvT": np.ascontiguousarray(wv_g.T).astype(ndt),
            "woT": np.ascontiguousarray(wo_g.T).astype(ndt),
            "cosT": cosT,
            "sinT": sinT,
            "mskT": msk.astype(ndt),
            "ones": np.ones((P, 1), dtype=ndt),
        })
    return in_maps


_CACHE = {}


def _get_program(mode):
    if mode not in _CACHE:
        _CACHE[mode] = build_program(mode)
    return _CACHE[mode]


def kernel(x, mask, wq, wk, wv, wo):
    mode = os.environ.get("BASS_ATTN_MODE", "f32r")
    nc = _get_program(mode)
    in_maps = host_prep(x, wq, wk, wv, wo, mode)
    res = run_bass_kernel_spmd(nc, in_maps, list(range(N_CORES))).results
    out = np.zeros((B, T, C), dtype=np.float32)
    for c in range(N_CORES):
        out[c // GROUP] += res[c]["y"]
    return out


# revision 69
# speedup vs baseline: 1.0046x; 1.0046x over previous
"""Causal GQA attention block (B=2, T=2048, C=2048, H=16, HKV=4, D=128, RoPE)
on 8 Trainium2 NeuronCores.

Sharding: core c handles batch b = c//4 and kv-group g = c%4 (4 q heads +
1 kv head per core).  The output projection is row-parallel: each core
produces a partial [T, C] contribution; the host sums the 4 partials per
batch.

Device-side design:
  - x is host-transposed to xT [C, T] so projections contract over the
    partition dim without any on-device transpose.
  - q/k are produced in head-transposed layout [D, T]; RoPE is applied
    there using a host-side even/odd permutation folded into wq/wk plus
    partition-swapped multiplies (out = q*cos + swap(q)*sin).
  - scores are computed transposed (S.T tiles [s, t]) so exp'd tiles feed
    the attn@v matmul directly as the moving operand; the softmax
    denominator comes from a ones-vector matmul accumulated alongside.
  - no max-subtraction in softmax: scores here are O(6), exp is safe in
    fp32, and the result is mathematically identical to the reference.
  - jt0 projections run k-outer (consuming streamed weight chunks), later
    chunks run output-major over an SBUF-cached x chunk set with a
    6-bank rotating PSUM pool so RoPE overlaps accumulation.

Matmul precision mode (env BASS_ATTN_MODE): "f32r" (default; fp32 storage,
float32r matmuls, ~1.4e-4 matmul rel err at full speed), "f32" (exact, 4x
slower), "bf16" (bf16 storage).
"""

import os
from contextlib import ExitStack

import numpy as np

import concourse.bass as bass
import concourse.tile as tile
from concourse import bacc, mybir
from concourse.bass_utils import run_bass_kernel_spmd
from concourse.masks import make_identity

# problem constants
B, T, C = 2, 2048, 2048
H, HKV, D = 16, 4, 128
GROUP = H // HKV           # 4 q heads per kv head
THETA = 1000000.0
SCALE = D ** -0.5

P = 128                    # partitions
TCH = 512                  # t-chunk (matmul moving free dim)
NJT = T // TCH             # 4 t-chunks
NK = C // P                # 16 contraction tiles
NH = GROUP                 # 4 local q heads
NST = T // P               # 16 s-tiles
N_CORES = 8

F32 = mybir.dt.float32


def _sb_dt(mode):
    if mode == "bf16":
        return mybir.dt.bfloat16
    if mode == "f32r":
        return mybir.dt.float32r
    return F32


def _np_dt(mode):
    if mode == "bf16":
        import ml_dtypes
        return ml_dtypes.bfloat16
    return np.float32


def build_program(mode="f32r", phases="ABC", variant=""):
    """Build and compile the per-core Bass program. Returns nc.

    phases/variant are diagnostic knobs for timeline bisection; production
    callers use the defaults.
    """
    sb_dt = _sb_dt(mode)

    nc = bacc.Bacc("TRN2", target_bir_lowering=False, debug=False)

    xT_d = nc.dram_tensor("xT", [C, T], sb_dt, kind="ExternalInput").ap()
    wq_d = nc.dram_tensor("wqT", [C, NH * D], sb_dt, kind="ExternalInput").ap()
    wk_d = nc.dram_tensor("wkT", [C, D], sb_dt, kind="ExternalInput").ap()
    wv_d = nc.dram_tensor("wvT", [C, D], sb_dt, kind="ExternalInput").ap()
    wo_d = nc.dram_tensor("woT", [NH * D, C], sb_dt, kind="ExternalInput").ap()
    cos_d = nc.dram_tensor("cosT", [P, T], F32, kind="ExternalInput").ap()
    sin_d = nc.dram_tensor("sinT", [P, T], F32, kind="ExternalInput").ap()
    msk_d = nc.dram_tensor("mskT", [P, 4 * TCH], sb_dt, kind="ExternalInput").ap()
    ones_d = nc.dram_tensor("ones", [P, 1], sb_dt, kind="ExternalInput").ap()
    y_d = nc.dram_tensor("y", [T, C], F32, kind="ExternalOutput").ap()

    with tile.TileContext(nc) as tc, ExitStack() as ctx:
        wpool = ctx.enter_context(tc.tile_pool(name="weights", bufs=1))
        tpool = ctx.enter_context(tc.tile_pool(name="tables", bufs=1))
        state = ctx.enter_context(tc.tile_pool(name="state", bufs=1))

        # wq in 2-k chunks on the scalar queue (consumed in k order by the
        # k-outer jt0 projections); wk/wv whole on gpsimd
        wq_sb = wpool.tile([P, NK, NH * D], sb_dt, tag="wq")
        wk_sb = wpool.tile([P, NK, D], sb_dt, tag="wk")
        wv_sb = wpool.tile([P, NK, D], sb_dt, tag="wv")
        for k2 in range(NK // 2):
            nc.scalar.dma_start(
                wq_sb[:, 2 * k2:2 * k2 + 2, :],
                wq_d[2 * k2 * P:(2 * k2 + 2) * P, :].rearrange(
                    "(ko p) o -> p ko o", p=P))
        nc.gpsimd.dma_start(wk_sb[:], wk_d.rearrange("(ko p) o -> p ko o", p=P))
        nc.gpsimd.dma_start(wv_sb[:], wv_d.rearrange("(ko p) o -> p ko o", p=P))

        cos_sb = tpool.tile([P, T], F32, tag="cos")
        sin_sb = tpool.tile([P, T], F32, tag="sin")
        for jt in range(NJT):
            sl = slice(jt * TCH, (jt + 1) * TCH)
            nc.gpsimd.dma_start(cos_sb[:, sl], cos_d[:, sl])
            nc.gpsimd.dma_start(sin_sb[:, sl], sin_d[:, sl])
        msk_sb = tpool.tile([P, 4 * TCH], sb_dt, tag="msk")
        nc.gpsimd.dma_start(msk_sb[:], msk_d[:])
        ones_sb = tpool.tile([P, 1], sb_dt, tag="ones")
        nc.gpsimd.dma_start(ones_sb[:], ones_d[:])
        ident_sb = tpool.tile([P, P], F32, tag="ident")
        make_identity(nc, ident_sb[:])

        qrot = state.tile([P, NH, T], sb_dt, tag="qrot")
        krot = state.tile([P, T], sb_dt, tag="krot")
        v_sb = state.tile([P, NST, D], sb_dt, tag="v")
        ot_sb = state.tile([P, NH, T], sb_dt, tag="ot")

        # ---------------- Phase A: projections + RoPE -----------------
        with ExitStack() as actx:
          if "A" in phases:
            xpool = actx.enter_context(tc.tile_pool(name="xsub", bufs=1))
            xpool2 = actx.enter_context(tc.tile_pool(name="xsub2", bufs=2))
            ropep = actx.enter_context(tc.tile_pool(name="rope", bufs=2))
            vtp = actx.enter_context(tc.tile_pool(name="vt", bufs=2))
            psA = actx.enter_context(tc.tile_pool(name="psA", bufs=1, space="PSUM"))
            psT = actx.enter_context(tc.tile_pool(name="psT", bufs=2, space="PSUM"))

            def rope(acc_ps, out_ap, jt):
                ch = slice(jt * TCH, (jt + 1) * TCH)
                m1 = ropep.tile([P, TCH], F32, tag="m1")
                m2 = ropep.tile([P, TCH], F32, tag="m2")
                nc.vector.tensor_tensor(
                    m1[:], acc_ps[:], cos_sb[:, ch], mybir.AluOpType.mult)
                nc.vector.tensor_tensor(
                    m2[0:64, :], acc_ps[64:128, :], sin_sb[0:64, ch],
                    mybir.AluOpType.mult)
                nc.vector.tensor_tensor(
                    m2[64:128, :], acc_ps[0:64, :], sin_sb[64:128, ch],
                    mybir.AluOpType.mult)
                nc.vector.tensor_tensor(
                    out_ap, m1[:], m2[:], mybir.AluOpType.add)

            def w_slice(o, k):
                # output index o: 0..3 = q heads, 4 = k, 5 = v
                if o < NH:
                    return wq_sb[:, k, o * D:(o + 1) * D]
                if o == NH:
                    return wk_sb[:, k, :]
                return wv_sb[:, k, :]

            def finish(o, acc, jt):
                if o < NH:
                    rope(acc, qrot[:, o, jt * TCH:(jt + 1) * TCH], jt)
                elif o == NH:
                    rope(acc, krot[:, jt * TCH:(jt + 1) * TCH], jt)
                else:
                    vt = vtp.tile([P, TCH], F32, tag="vt")
                    nc.vector.tensor_copy(vt[:], acc[:])
                    for i in range(TCH // P):
                        pst = psT.tile([P, P], F32, tag="pst")
                        nc.tensor.transpose(pst[:], vt[:, i * P:(i + 1) * P],
                                            ident_sb[:])
                        nc.scalar.copy(v_sb[:, jt * (TCH // P) + i, :], pst[:])

            nacc = 0  # rotating psum bank index
            for jt in range(NJT):
                xs = []
                for k in range(NK):
                    pool_k = xpool2 if k < 5 else xpool
                    xt = pool_k.tile([P, TCH], sb_dt, tag=f"x{k}")
                    nc.sync.dma_start(
                        xt[:],
                        xT_d[k * P:(k + 1) * P, jt * TCH:(jt + 1) * TCH])
                    xs.append(xt)

                if jt == 0:
                    # k-outer: consume weight chunks as they stream in
                    accs = [psA.tile([P, TCH], F32, tag=f"acc{o}", name=f"acc{o}")
                            for o in range(6)]
                    for k in range(NK):
                        for o in range(6):
                            nc.tensor.matmul(
                                accs[o][:], w_slice(o, k), xs[k][:],
                                start=(k == 0), stop=(k == NK - 1))
                    for o in (5, 4, 0, 1, 2, 3):
                        finish(o, accs[o], jt)
                    nacc = 6
                else:
                    # output-major: RoPE of one output overlaps the next
                    # output's accumulation via the rotating bank pool
                    for o in range(6):
                        acc = psA.tile([P, TCH], F32, tag=f"acc{nacc % 6}",
                                       name=f"accr{nacc % 6}")
                        nacc += 1
                        for k in range(NK):
                            nc.tensor.matmul(
                                acc[:], w_slice(o, k), xs[k][:],
                                start=(k == 0), stop=(k == NK - 1))
                        finish(o, acc, jt)

        # ---------------- Phase B: attention ---------------------------
        # prefetch the output-projection weights during attention so phase C
        # starts without a DMA stall
        wopool = ctx.enter_context(tc.tile_pool(name="wo", bufs=1))
        wo_sb = wopool.tile([P, NH, C], sb_dt, tag="wo")
        for h in range(NH):
            nc.scalar.dma_start(wo_sb[:, h, :], wo_d[h * P:(h + 1) * P, :])

        with ExitStack() as bctx:
          if "B" in phases:
            esp = bctx.enter_context(tc.tile_pool(name="es", bufs=5))
            rcp = bctx.enter_context(tc.tile_pool(name="rc", bufs=2))
            psS = bctx.enter_context(tc.tile_pool(name="psS", bufs=4, space="PSUM"))
            psO = bctx.enter_context(tc.tile_pool(name="psO", bufs=3, space="PSUM"))
            psD = bctx.enter_context(tc.tile_pool(name="psD", bufs=1, space="PSUM"))

            for jt in range(NJT):
                for h in range(NH):
                    njs = 4 * jt + 4
                    ot_ps = psO.tile([P, TCH], F32, tag="ot")
                    dn_ps = psD.tile([1, TCH], F32, tag="dn")
                    qch = qrot[:, h, jt * TCH:(jt + 1) * TCH]

                    def emit_pv(es, js, njs=njs, ot_ps=ot_ps, dn_ps=dn_ps):
                        nc.tensor.matmul(
                            ot_ps[:], v_sb[:, js, :], es[:],
                            start=(js == 0), stop=(js == njs - 1))
                        if variant != "noden":
                            nc.tensor.matmul(
                                dn_ps[:], ones_sb[:, 0:1], es[:],
                                start=(js == 0), stop=(js == njs - 1))

                    pend = []  # deferred two steps to hide exp latency
                    for js in range(njs):
                        s_ps = psS.tile([P, TCH], F32, tag="s")
                        nc.tensor.matmul(
                            s_ps[:], krot[:, js * P:(js + 1) * P], qch,
                            start=True, stop=True)
                        es = esp.tile([P, TCH], sb_dt, tag="es")
                        nc.scalar.activation(
                            es[:], s_ps[:], mybir.ActivationFunctionType.Exp)
                        if js // 4 == jt:
                            nc.vector.tensor_tensor(
                                es[:], es[:],
                                msk_sb[:, (js % 4) * TCH:(js % 4 + 1) * TCH],
                                mybir.AluOpType.mult)
                        if len(pend) >= 3:
                            emit_pv(*pend.pop(0))
                        pend.append((es, js))
                    for p_ in pend:
                        emit_pv(*p_)

                    if variant == "noden":
                        nc.vector.tensor_copy(
                            ot_sb[:, h, jt * TCH:(jt + 1) * TCH], ot_ps[:])
                    else:
                        rec = rcp.tile([1, TCH], F32, tag="rec")
                        nc.vector.reciprocal(rec[:], dn_ps[:])
                        rb = rcp.tile([P, TCH], F32, tag="rb")
                        nc.gpsimd.partition_broadcast(rb[:], rec[:])
                        nc.vector.tensor_tensor(
                            ot_sb[:, h, jt * TCH:(jt + 1) * TCH], ot_ps[:],
                            rb[:], mybir.AluOpType.mult)

        # ---------------- Phase C: output projection -------------------
        with ExitStack() as cctx:
          if "C" in phases:
            ypool = cctx.enter_context(tc.tile_pool(name="ysb", bufs=8))
            psC = cctx.enter_context(tc.tile_pool(name="psC", bufs=4, space="PSUM"))

            for tt in range(NST):
                for jc in range(NJT):
                    yp = psC.tile([P, TCH], F32, tag="y")
                    for h in range(NH):
                        nc.tensor.matmul(
                            yp[:],
                            ot_sb[:, h, tt * P:(tt + 1) * P],
                            wo_sb[:, h, jc * TCH:(jc + 1) * TCH],
                            start=(h == 0), stop=(h == NH - 1))
                    ys = ypool.tile([P, TCH], F32, tag="ys")
                    nc.vector.tensor_copy(ys[:], yp[:])
                    nc.sync.dma_start(
                        y_d[tt * P:(tt + 1) * P, jc * TCH:(jc + 1) * TCH],
                        ys[:])

    nc.compile()
    return nc


def host_prep(x, wq, wk, wv, wo, mode="f32r"):
    """Build the 8 per-core input maps (numpy, host-side reshuffles only)."""
    ndt = _np_dt(mode)
    x = np.asarray(x, dtype=np.float32)
    wq = np.asarray(wq, dtype=np.float32)
    wk = np.asarray(wk, dtype=np.float32)
    wv = np.asarray(wv, dtype=np.float32)
    wo = np.asarray(wo, dtype=np.float32)

    # RoPE even/odd grouping permutation within each head
    perm = np.concatenate([np.arange(0, D, 2), np.arange(1, D, 2)])

    # rope tables, transposed layout [d, t], matching reference f32 math
    inv_freq = (1.0 / THETA ** (np.arange(0, D, 2, dtype=np.float32) / D)).astype(np.float32)
    pos = np.arange(T, dtype=np.float32)
    freqs = pos[:, None] * inv_freq[None, :]          # [T, 64] f32
    cos_t = np.cos(freqs).astype(np.float32).T        # [64, T]
    sin_t = np.sin(freqs).astype(np.float32).T        # [64, T]
    cosT = np.concatenate([cos_t, cos_t], axis=0)     # [128, T]
    sinT = np.concatenate([-sin_t, sin_t], axis=0)    # [128, T]

    # diagonal-block causal masks (multiplicative, after exp)
    # pattern r (= js % 4): allow f >= 128*r + p
    f = np.arange(TCH)[None, :]
    p = np.arange(P)[:, None]
    msk = np.concatenate(
        [(f >= P * r + p).astype(np.float32) for r in range(4)], axis=1)

    xTs = [np.ascontiguousarray(x[b].T).astype(ndt) for b in range(B)]

    in_maps = []
    for c in range(N_CORES):
        b, g = divmod(c, GROUP)
        rows = []
        for hh in range(NH):
            h = g * GROUP + hh
            rows.append(wq[h * D + perm, :])
        wq_g = np.concatenate(rows, axis=0) * SCALE          # [512, C]
        wk_g = wk[g * D + perm, :]                           # [128, C]
        wv_g = wv[g * D:(g + 1) * D, :]                      # [128, C]
        wo_g = wo[:, g * NH * D:(g + 1) * NH * D]            # [C, 512]

        in_maps.append({
            "xT": xTs[b],
            "wqT": np.ascontiguousarray(wq_g.T).astype(ndt),
            "wkT": np.ascontiguousarray(wk_g.T).astype(ndt),
            "wvT": np.ascontiguousarray(wv_g.T).astype(ndt),
            "woT": np.ascontiguousarray(wo_g.T).astype(ndt),
            "cosT": cosT,
            "sinT": sinT,
            "mskT": msk.astype(ndt),
            "ones": np.ones((P, 1), dtype=ndt),
        })
    return in_maps


_CACHE = {}


def _get_program(mode):
    if mode not in _CACHE:
        _CACHE[mode] = build_program(mode)
    return _CACHE[mode]


def kernel(x, mask, wq, wk, wv, wo):
    mode = os.environ.get("BASS_ATTN_MODE", "f32r")
    nc = _get_program(mode)
    in_maps = host_prep(x, wq, wk, wv, wo, mode)
    res = run_bass_kernel_spmd(nc, in_maps, list(range(N_CORES))).results
    out = np.zeros((B, T, C), dtype=np.float32)
    for c in range(N_CORES):
        out[c // GROUP] += res[c]["y"]
    return out
